# revision 1
# baseline (speedup 1.0000x reference)
"""2D-DCT (DCT-II, orthonormal) spatial transform on Trainium2, 8 NeuronCores.

Full input x [16,256,128,128] f32 -> out[b,c,k,v] = sum_hw Wy[k,h] Wx[v,w] x[b,c,h,w]
with Wy = Wx = 128-point orthonormal DCT-II matrix W.

Strategy (data-parallel, batch*channel sharded 4096 -> 512 images/core):
per image X: out = W @ X @ W.T via two PE matmuls and zero explicit
transposes -- matmul(out, lhsT, rhs) = lhsT.T @ rhs transposes the
stationary operand for free:
  mm1: lhsT=X_i   (fp16), rhs=W.T (fp16) -> Z^T = (W@X)^T   (f32 PSUM)
  mm2: lhsT=Z^T_i (fp16), rhs=W.T (fp16) -> out_i           (f32 PSUM)

The kernel is HBM-bound (in+out traffic is the floor), so I/O is fp16:
the host rounds x to fp16 (eps 4.9e-4 << the 2e-2 tolerance; measured
end-to-end error ~1e-3) and repacks each group of 8 images to
[g, h, i, w] so every DMA descriptor element is one contiguous 2 KB
per-partition run (>=512 B keeps SDMA at line rate). That halves HBM
traffic per image to 64 KB (~190 ns/img/core roofline vs ~366 for f32).
fp16 matmuls run 1 PE cycle/row (vs 4 for fp32) and get the compiler's
Fast Weight Load, so PE (~2x128 cycles + 2 LDW per image) stays under
the DMA roofline, as do the batched PSUM->SBUF cast copies (stage 1 on
ACT, stage 2 on DVE). Accumulation is f32 in PSUM; the f32 output dtype
is restored on the host.
"""

import sys

for _p in ("/opt/trn_rl_repo", "/root/.axon_site/_ro/trn_rl_repo"):
    if _p not in sys.path:
        sys.path.insert(0, _p)

import numpy as np

N_CORES = 8
B, C, H, W = 16, 256, 128, 128
PER_CORE = B * C // N_CORES  # 512 images per core
GROUP = 8                    # images per DMA group (baked into DRAM layout)


def _dct_matrix(n: int) -> np.ndarray:
    v = np.arange(n, dtype=np.float64)[:, None]
    j = np.arange(n, dtype=np.float64)[None, :]
    f = np.cos(np.pi * (0.5 + j) * v / n) / np.sqrt(n)
    f *= np.where(v != 0, np.sqrt(2.0), 1.0)
    return f.astype(np.float32)


def _build_program(n_img: int, group: int = GROUP, xg_bufs: int = 4,
                   og_bufs: int = 4, p1_bufs: int = 2, p2_bufs: int = 2,
                   zt_bufs: int = 3, reps: int = 1):
    import contextlib

    import concourse.bacc as bacc_mod
    import concourse.mybir as mybir
    from concourse.tile import TileContext

    F32 = mybir.dt.float32
    F16 = mybir.dt.float16

    ng = n_img // group
    nc = bacc_mod.Bacc()
    # packed input/output: [group-idx, row, img-in-group, col] fp16 so each
    # partition's DMA element is one contiguous 2 KB run
    x = nc.declare_dram_parameter("x", [ng, 128, group, 128], F16, isOutput=False)
    wt_p = nc.declare_dram_parameter("wt", [128, 128], F16, isOutput=False)
    out = nc.declare_dram_parameter("out", [ng, 128, group, 128], F16, isOutput=True)

    with TileContext(nc) as tc:
        with tc.tile_pool(name="consts", bufs=1) as cpool, \
             tc.tile_pool(name="xin", bufs=xg_bufs) as xpool, \
             tc.tile_pool(name="mid", bufs=zt_bufs) as zpool, \
             tc.tile_pool(name="oput", bufs=og_bufs) as opool, \
             tc.tile_pool(name="ps", bufs=1, space="PSUM") as pspool:
            wt = cpool.tile([128, 128], F16)
            nc.sync.dma_start(out=wt, in_=wt_p[:])

            # PE warm-up dummy: absorbs the wt-DMA wait so no later
            # (self-loading) matmul needs more than one sync wait -- the
            # S3_LW struct can carry only one. Writes into the p2 rotation.
            pdum = pspool.tile([128, group, 128], F32, tag="p2", bufs=p2_bufs)
            nc.tensor.matmul(pdum[:, 0, :], lhsT=wt, rhs=wt,
                             start=True, stop=True)

            loop_ctx = tc.For_i(0, reps) if reps > 1 else contextlib.nullcontext()
            with loop_ctx:
                for g in range(ng):
                    xg = xpool.tile([128, group, 128], F16, tag="xg")
                    nc.sync.dma_start(out=xg, in_=x[g])
                    p1 = pspool.tile([128, group, 128], F32, tag="p1",
                                     bufs=p1_bufs)
                    for i in range(group):
                        nc.tensor.matmul(p1[:, i, :], lhsT=xg[:, i, :],
                                         rhs=wt, start=True, stop=True)
                    zt = zpool.tile([128, group, 128], F16, tag="zt")
                    nc.scalar.copy(out=zt, in_=p1)  # batched f32->fp16 (ACT)
                    p2 = pspool.tile([128, group, 128], F32, tag="p2",
                                     bufs=p2_bufs)
                    for i in range(group):
                        nc.tensor.matmul(p2[:, i, :], lhsT=zt[:, i, :],
                                         rhs=wt, start=True, stop=True)
                    og = opool.tile([128, group, 128], F16, tag="og")
                    nc.vector.tensor_copy(out=og, in_=p2)  # f32->fp16 (DVE)
                    nc.sync.dma_start(out=out[g], in_=og)
    nc.finalize()
    return nc


_CACHE = {}


def kernel(x: np.ndarray) -> np.ndarray:
    from concourse.bass_utils import run_bass_kernel_spmd

    assert x.shape == (B, C, H, W), x.shape

    if "nc" not in _CACHE:
        _CACHE["nc"] = _build_program(PER_CORE)
    nc = _CACHE["nc"]

    wt = np.ascontiguousarray(_dct_matrix(128).T).astype(np.float16)

    # host-side pack: f32 -> fp16, [512,128,128] -> [64,128,8,128] per core
    flat = np.asarray(x, dtype=np.float16).reshape(B * C, H, W)
    ng = PER_CORE // GROUP
    in_maps = []
    for c in range(N_CORES):
        shard = flat[c * PER_CORE:(c + 1) * PER_CORE]
        packed = np.ascontiguousarray(
            shard.reshape(ng, GROUP, H, W).transpose(0, 2, 1, 3))
        in_maps.append({"x": packed, "wt": wt})

    res = run_bass_kernel_spmd(nc, in_maps, list(range(N_CORES)))
    outs = []
    for r in res.results:
        o = r["out"]  # [64, 128, 8, 128] fp16 = [g, k, i, v]
        outs.append(o.transpose(0, 2, 1, 3).reshape(PER_CORE, H, W))
    out = np.concatenate(outs, axis=0).astype(np.float32)
    return out.reshape(B, C, H, W)


if __name__ == "__main__":
    rng = np.random.default_rng(0)
    xs = rng.standard_normal((B, C, H, W), dtype=np.float32)
    o = kernel(xs)
    print("kernel output", o.shape, o.dtype)



# revision 2
# speedup vs baseline: 1.9782x; 1.9782x over previous
"""2D-DCT (DCT-II, orthonormal) spatial transform on Trainium2, 8 NeuronCores.

Full input x [16,256,128,128] f32 -> out[b,c,k,v] = sum_hw Wy[k,h] Wx[v,w] x[b,c,h,w]
with Wy = Wx = 128-point orthonormal DCT-II matrix W.

Strategy (data-parallel, batch*channel sharded 4096 -> 512 images/core):
per image X: out = W @ X @ W.T on the PE array with zero explicit
transposes -- matmul(out, lhsT, rhs) = lhsT.T @ rhs transposes the
stationary operand for free:
  mm1 (per image):  lhsT=X_i (fp16), rhs=W.T -> Z_i^T = (W@X_i)^T  (f32 PSUM)
  mm2 (per 4 imgs): lhsT=W.T stationary, rhs=Z^T batch -> out_i^T   (f32 PSUM)
so only mm1 reloads PE weights per image; mm2 streams 512-wide against
the resident DCT matrix.

The kernel is HBM-bound (fp16 in+out is the floor: 64 KB/img, ~214
ns/img/core measured for pure DMA under full 8-core load), so the
structure is built around DMA efficiency:
- fp16 I/O (eps 4.9e-4 << the 2e-2 tolerance; measured end-to-end
  error ~5e-4), f32 accumulation in PSUM; f32 restored on host.
- 32-image DMA groups: host repacks to [g, h, i, w] so each transfer
  is 1 MB with 8 KB contiguous per partition (one descriptor per
  partition at SDMA line rate); only 16+16 dma_start instructions per
  core amortize the ~630 ns/instr HWDGE descriptor-gen serialization
  that throttled finer-grained grouping.
- input DMAs issue from SP (qSP HWDGE queue), output DMAs from the
  Activation engine (qACT HWDGE queue) so in/out descriptor generation
  runs in parallel.
- compute per 8-image PSUM sub-group: 8x mm1, batched ACT cast
  PSUM->fp16 SBUF, 2x wide mm2, batched DVE cast into the 32-image
  output tile. PE/ACT/DVE each stay under the DMA stage time, and
  4-deep input/output tile rotation keeps the DMA queues busy.
"""

import sys

for _p in ("/opt/trn_rl_repo", "/root/.axon_site/_ro/trn_rl_repo"):
    if _p not in sys.path:
        sys.path.insert(0, _p)

import numpy as np

N_CORES = 8
B, C, H, W = 16, 256, 128, 128
PER_CORE = B * C // N_CORES  # 512 images per core
DMA_GROUP = 32               # images per DMA transfer (baked into DRAM layout)
SUB = 8                      # images per PSUM compute sub-group


def _dct_matrix(n: int) -> np.ndarray:
    v = np.arange(n, dtype=np.float64)[:, None]
    j = np.arange(n, dtype=np.float64)[None, :]
    f = np.cos(np.pi * (0.5 + j) * v / n) / np.sqrt(n)
    f *= np.where(v != 0, np.sqrt(2.0), 1.0)
    return f.astype(np.float32)


def _build_program(n_img: int, dma_group: int = DMA_GROUP, sub: int = SUB,
                   xg_bufs: int = 4, og_bufs: int = 4, zt_bufs: int = 4,
                   p1_bufs: int = 2, p2_bufs: int = 2, reps: int = 1):
    import contextlib

    import concourse.bacc as bacc_mod
    import concourse.mybir as mybir
    from concourse.tile import TileContext

    F32 = mybir.dt.float32
    F16 = mybir.dt.float16

    ng = n_img // dma_group
    nsub = dma_group // sub
    nc = bacc_mod.Bacc()
    # packed input/output: [group-idx, row, img-in-group, col] fp16 so each
    # partition's DMA element is one contiguous 8 KB run
    x = nc.declare_dram_parameter("x", [ng, 128, dma_group, 128], F16,
                                  isOutput=False)
    wt_p = nc.declare_dram_parameter("wt", [128, 128], F16, isOutput=False)
    out = nc.declare_dram_parameter("out", [ng, 128, dma_group, 128], F16,
                                    isOutput=True)

    with TileContext(nc) as tc:
        with tc.tile_pool(name="consts", bufs=1) as cpool, \
             tc.tile_pool(name="xin", bufs=xg_bufs) as xpool, \
             tc.tile_pool(name="mid", bufs=zt_bufs) as zpool, \
             tc.tile_pool(name="oput", bufs=og_bufs) as opool, \
             tc.tile_pool(name="ps", bufs=1, space="PSUM") as pspool:
            wt = cpool.tile([128, 128], F16)
            nc.sync.dma_start(out=wt, in_=wt_p[:])

            # PE warm-up dummy: absorbs the wt-DMA wait so no later
            # (self-loading) matmul needs more than one sync wait -- the
            # S3_LW struct can carry only one. Writes into the p2 rotation.
            pdum = pspool.tile([128, sub, 128], F32, tag="p2", bufs=p2_bufs)
            nc.tensor.matmul(pdum[:, 0, :], lhsT=wt, rhs=wt,
                             start=True, stop=True)

            loop_ctx = tc.For_i(0, reps) if reps > 1 else contextlib.nullcontext()
            with loop_ctx:
                for g in range(ng):
                    xg = xpool.tile([128, dma_group, 128], F16, tag="xg")
                    nc.sync.dma_start(out=xg, in_=x[g])
                    og = opool.tile([128, dma_group, 128], F16, tag="og")
                    for s in range(nsub):
                        p1 = pspool.tile([128, sub, 128], F32, tag="p1",
                                         bufs=p1_bufs)
                        for i in range(sub):
                            nc.tensor.matmul(p1[:, i, :],
                                             lhsT=xg[:, s * sub + i, :],
                                             rhs=wt, start=True, stop=True)
                        zt = zpool.tile([128, sub, 128], F16, tag="zt")
                        nc.scalar.copy(out=zt, in_=p1)  # f32->fp16 (ACT)
                        p2 = pspool.tile([128, sub, 128], F32, tag="p2",
                                         bufs=p2_bufs)
                        # out_i^T = W @ Z_i^T, streamed against stationary
                        # wt; split so each matmul's output stays within
                        # one PSUM bank (512 f32 per partition).
                        half = sub // 2
                        for h in range(2):
                            nc.tensor.matmul(
                                p2[:, h * half:(h + 1) * half, :],
                                lhsT=wt,
                                rhs=zt[:, h * half:(h + 1) * half, :],
                                start=True, stop=True)
                        # f32->fp16 (DVE) into the 32-image output tile
                        nc.vector.tensor_copy(
                            out=og[:, s * sub:(s + 1) * sub, :], in_=p2)
                    nc.scalar.dma_start(out=out[g], in_=og)
    nc.finalize()
    return nc


_CACHE = {}


def kernel(x: np.ndarray) -> np.ndarray:
    from concourse.bass_utils import run_bass_kernel_spmd

    assert x.shape == (B, C, H, W), x.shape

    if "nc" not in _CACHE:
        _CACHE["nc"] = _build_program(PER_CORE)
    nc = _CACHE["nc"]

    wt = np.ascontiguousarray(_dct_matrix(128).T).astype(np.float16)

    # host-side pack: f32 -> fp16, [512,128,128] -> [16,128,32,128] per core
    flat = np.asarray(x, dtype=np.float16).reshape(B * C, H, W)
    ng = PER_CORE // DMA_GROUP
    in_maps = []
    for c in range(N_CORES):
        shard = flat[c * PER_CORE:(c + 1) * PER_CORE]
        packed = np.ascontiguousarray(
            shard.reshape(ng, DMA_GROUP, H, W).transpose(0, 2, 1, 3))
        in_maps.append({"x": packed, "wt": wt})

    res = run_bass_kernel_spmd(nc, in_maps, list(range(N_CORES)))
    outs = []
    for r in res.results:
        o = r["out"]  # [16, 128, 32, 128] fp16 = [g, u, i, v]
        # mm2 leaves partition dim = width-freq u, free = height-freq v
        outs.append(o.transpose(0, 2, 3, 1).reshape(PER_CORE, H, W))
    out = np.concatenate(outs, axis=0).astype(np.float32)
    return out.reshape(B, C, H, W)


if __name__ == "__main__":
    rng = np.random.default_rng(0)
    xs = rng.standard_normal((B, C, H, W), dtype=np.float32)
    o = kernel(xs)
    print("kernel output", o.shape, o.dtype)
